# revision 33
# baseline (speedup 1.0000x reference)
"""Trainium2 Bass kernel for nn_Decoder2 (GRU decoder, Keras reset_after GRUCell).

Reference computation (per batch row b, scanned over t = 0..T-1):
    x_t   = [o_{t-1}, feat_t]                  # [1+F]
    mx    = x_t @ K + ib                       # [3H]
    mh    = h_{t-1} @ Wr + rb                  # [3H]
    z     = sigmoid(mx[:H]   + mh[:H])
    r     = sigmoid(mx[H:2H] + mh[H:2H])
    cand  = tanh(mx[2H:] + r * mh[2H:])
    h_t   = z * h_{t-1} + (1-z) * cand
    o_t   = h_t @ dw + db                      # scalar output per row

Shapes: B=8192, T=96, F=64, H=256.  Data parallel over batch: 1024 rows
per core on 8 cores, no collectives.

Design notes (see git history of this file for the evolution):
  * o-feedback folded off the critical path: z/r via Wr' = Wr + dw@k0^T
    (host-side), the h gate via a K=1 matmul against o_sb(t-1) (one step
    of slack).  db is handled exactly via o_raw = o - db and a host-side
    add on the output.  x tiles carry only the 64 feature rows.
  * F=64 == 128/2, so all feature matmuls (K=64) are row-tile PAIRED:
    chunk 0 runs in PE rows 64..127 against a duplicated copy of the
    features, chunk 1 in rows 0..63 - two matmuls per pair slot.  The two
    K=1 o-matmuls pair the same way via an o copy at partition 32 (made
    by a free SBUF->SBUF DMA).
  * optional per-gate fp8 e4m3 DoubleRow for the recurrent matmuls
    (gates8 string, subset of "zrh"; 'h' = the hh gate): one K=256 matmul
    per gate chunk instead of two.  fp8 weights are pre-scaled by S=16
    (the 1/S rides the activation's free scale operand).  Error impact
    (measured numpy sim + HW): r alone is numerically free; z is the
    sensitive gate.  Requires an fp8 copy of h (double-emitted add).
  * optional ci_pe: xh += r*hh via identity-weight matmuls accumulating
    into the open xh PSUM group - removes a 1x-rate PSUM-source DVE pass;
    pays off when fp8 gives the tensor engine headroom.
  * per-block software pipelining: block (t, j) carries the chain tail of
    the previous block, split in two (ci+tanh early, combine after this
    block's rh) so no engine head-of-line blocks another's latency.
  * GPSIMD is deliberately unused: it shares an SBUF port with VectorE
    and degrades every DVE op to 1x rate (measured).

PSUM (8 banks): pzr 2 (r then z sequentially), phh/dense-o 2 (shared
slot), pxh 2x2 (double-buffered across blocks).
"""

import os
import sys

for _p in ("/root/.axon_site/_ro/trn_rl_repo", "/opt/trn_rl_repo"):
    if os.path.isdir(_p) and _p not in sys.path:
        sys.path.insert(0, _p)

from contextlib import ExitStack  # noqa: E402

import numpy as np  # noqa: E402

import concourse.bacc as bacc  # noqa: E402
import concourse.tile as tile  # noqa: E402
from concourse import mybir  # noqa: E402
from concourse import bass_utils  # noqa: E402

Alu = mybir.AluOpType
Act = mybir.ActivationFunctionType
PerfMode = mybir.MatmulPerfMode

B, T, F, H = 8192, 96, 64, 256
G3 = 3 * H
NCORES = 8
BL = B // NCORES
NCH = 2                      # 128-row chunks of H
S8 = 16.0                    # fp8 weight pre-scale


def build_nc(
    t_steps: int = T,
    bl: int = BL,
    nt: int = 2,
    gates8: str = "",
    ocast_act: bool = True,
    ci_pe: bool = False,
):
    """Build + compile the per-core Bass program.

    gates8:    subset of "zrh": which recurrent matmuls use fp8 DoubleRow
               ('h' = the hh gate).  "" = all bf16.
    ocast_act: dense-o PSUM->SBUF cast on ScalarE (else VectorE)
    ci_pe:     xh += r*hh via identity matmul on TensorE (else in-place
               DVE add)
    """
    n = bl // nt
    assert n <= 512
    f32 = mybir.dt.float32
    bf = mybir.dt.bfloat16
    f8 = mybir.dt.float8e4
    any8 = bool(gates8)
    z8, r8, hh8 = ("z" in gates8), ("r" in gates8), ("h" in gates8)
    sz = 1.0 / S8 if z8 else 1.0     # ACT scale compensations
    sr = 1.0 / S8 if r8 else 1.0
    sh = 1.0 / S8 if hh8 else 1.0

    nc = bacc.Bacc("TRN2", target_bir_lowering=False, debug=False)

    featT2 = nc.dram_tensor("featT2", [t_steps, 128, bl], bf, kind="ExternalInput").ap()
    h0T = nc.dram_tensor("h0T", [128, NCH, bl], bf, kind="ExternalInput").ap()
    h08T = nc.dram_tensor("h08T", [128, NCH, bl], f8, kind="ExternalInput").ap()
    o0 = nc.dram_tensor("o0", [1, bl], bf, kind="ExternalInput").ap()
    # x-side weights (columns pre-scaled to match each gate's weight
    # scale): rows 0..63 = kernel[1:], rows 64..127 duplicate
    kxf = nc.dram_tensor("kxf", [128, G3], bf, kind="ExternalInput").ap()
    # k0 row (same column scaling) at partitions 0 and 32
    k0t = nc.dram_tensor("k0t", [33, G3], bf, kind="ExternalInput").ap()
    # recurrent weights (folded z/r cols), [ki, ko, col]: bf16 and fp8(x16)
    wrqb = nc.dram_tensor("wrqb", [128, NCH, G3], bf, kind="ExternalInput").ap()
    wrq8 = nc.dram_tensor("wrq8", [128, NCH, G3], f8, kind="ExternalInput").ap()
    # unfolded z/r columns for t=0 (o_{-1} is the external init input)
    wrz0b = nc.dram_tensor("wrz0b", [128, NCH, 2 * H], bf, kind="ExternalInput").ap()
    wrz08 = nc.dram_tensor("wrz08", [128, NCH, 2 * H], f8, kind="ExternalInput").ap()
    dww = nc.dram_tensor("dww", [128, NCH], bf, kind="ExternalInput").ap()
    ident = nc.dram_tensor("ident", [128, 128], bf, kind="ExternalInput").ap()
    outT = nc.dram_tensor("outT", [t_steps, bl], bf, kind="ExternalOutput").ap()

    with tile.TileContext(nc) as tc, ExitStack() as ctx:
        const = ctx.enter_context(tc.tile_pool(name="const", bufs=1))
        hpool = ctx.enter_context(tc.tile_pool(name="h", bufs=3))
        h8pool = ctx.enter_context(tc.tile_pool(name="h8", bufs=3))
        xpool = ctx.enter_context(tc.tile_pool(name="x", bufs=6))
        rpool = ctx.enter_context(tc.tile_pool(name="rsb", bufs=3))
        zpool = ctx.enter_context(tc.tile_pool(name="zsb", bufs=3))
        rhpool = ctx.enter_context(tc.tile_pool(name="rh", bufs=3))
        cpool = ctx.enter_context(tc.tile_pool(name="cand", bufs=3))
        dpool = ctx.enter_context(tc.tile_pool(name="dsb", bufs=3))
        epool = ctx.enter_context(tc.tile_pool(name="esb", bufs=3))
        opool = ctx.enter_context(tc.tile_pool(name="osb", bufs=4))
        pzr = ctx.enter_context(tc.tile_pool(name="pzr", bufs=1, space="PSUM"))
        phh = ctx.enter_context(tc.tile_pool(name="phh", bufs=1, space="PSUM"))
        pxh = ctx.enter_context(tc.tile_pool(name="pxh", bufs=2, space="PSUM"))

        # --- constants ---
        kxf_sb = const.tile([128, G3], bf)
        nc.sync.dma_start(out=kxf_sb, in_=kxf)
        k0t_sb = const.tile([33, G3], bf)
        nc.sync.dma_start(out=k0t_sb, in_=k0t)
        wrb_sb = const.tile([128, NCH, G3], bf)
        nc.sync.dma_start(out=wrb_sb, in_=wrqb)
        wrz0b_sb = const.tile([128, NCH, 2 * H], bf)
        nc.sync.dma_start(out=wrz0b_sb, in_=wrz0b)
        if any8:
            wr8_sb = const.tile([128, NCH, G3], f8)
            nc.sync.dma_start(out=wr8_sb, in_=wrq8)
            wrz08_sb = const.tile([128, NCH, 2 * H], f8)
            nc.sync.dma_start(out=wrz08_sb, in_=wrz08)
        dw_sb = const.tile([128, NCH], bf)
        nc.sync.dma_start(out=dw_sb, in_=dww)
        id_sb = const.tile([128, 128], bf)
        nc.sync.dma_start(out=id_sb, in_=ident)
        # o rows at partitions 0 and 32 (K=1 o-matmul row-tile pairing)
        o0_sb = const.tile([33, bl], bf)
        nc.sync.dma_start(out=o0_sb[0:1, :], in_=o0)
        nc.sync.dma_start(out=o0_sb[32:33, :], in_=o0)

        # --- initial state ---
        h_prev = hpool.tile([128, NCH, bl], bf)
        nc.sync.dma_start(out=h_prev, in_=h0T)
        h8_prev = h_prev
        if any8:
            h8_prev = h8pool.tile([128, NCH, bl], f8)
            nc.sync.dma_start(out=h8_prev, in_=h08T)
        xs = {}
        for j in range(nt):
            xj = xpool.tile([128, n], bf, tag="x")
            nc.sync.dma_start(out=xj, in_=featT2[0, :, j * n:(j + 1) * n])
            xs[(0, j)] = xj

        def h_mms(gp, gcol, t, bs, use8):
            """Recurrent matmuls for one 128-wide gate chunk at column
            gcol.  Feat matmuls opened the group; closes it unless t==0
            (the K=1 o0 matmul closes it then)."""
            t0zr = t == 0 and gcol < 2 * H
            if use8:
                w = wrz08_sb if t0zr else wr8_sb
                nc.tensor.matmul(gp, w[:, :, gcol:gcol + 128],
                                 h8_prev[:, :, bs], start=False, stop=t != 0,
                                 perf_mode=PerfMode.DoubleRow)
            else:
                w = wrz0b_sb if t0zr else wrb_sb
                nc.tensor.matmul(gp, w[:, 0, gcol:gcol + 128],
                                 h_prev[:, 0, bs], start=False, stop=False)
                nc.tensor.matmul(gp, w[:, 1, gcol:gcol + 128],
                                 h_prev[:, 1, bs], start=False, stop=t != 0)
            if t == 0:
                nc.tensor.matmul(gp, k0t_sb[0:1, gcol:gcol + 128],
                                 o0_sb[0:1, bs], start=False, stop=True)

        # chain-tail state carried between blocks
        pending = None
        pending_head = None

        def emit_tail_head():
            """First half of the previous block's tail: ci + tanh."""
            nonlocal pending, pending_head
            if pending is None:
                return
            pbs, xhp, rh_sb, zsb, hprv, hnew, h8new = pending
            pending = None
            if ci_pe:
                nc.tensor.matmul(xhp[:, 0, :], id_sb, rh_sb[:, 0, :],
                                 start=False, stop=True)
                nc.tensor.matmul(xhp[:, 1, :], id_sb, rh_sb[:, 1, :],
                                 start=False, stop=True)
            else:
                nc.vector.tensor_tensor(xhp, xhp, rh_sb, Alu.add)
            cand = cpool.tile([128, NCH, n], bf, tag="cand")
            nc.scalar.activation(cand, xhp, Act.Tanh, scale=sh)
            pending_head = (pbs, cand, zsb, hprv, hnew, h8new)

        def emit_tail_combine():
            """Second half: h_new = cand + z*(h_prev - cand), emitted after
            this block's rh so rh heads the DVE queue."""
            nonlocal pending_head
            if pending_head is None:
                return
            pbs, cand, zsb, hprv, hnew, h8new = pending_head
            pending_head = None
            d_sb = dpool.tile([128, NCH, n], bf, tag="dsb")
            nc.vector.tensor_tensor(d_sb, hprv[:, :, pbs], cand, Alu.subtract)
            e_sb = epool.tile([128, NCH, n], bf, tag="esb")
            nc.vector.tensor_tensor(e_sb, zsb, d_sb, Alu.mult)
            if any8:
                # fp8 copy first: it feeds the next step's matmuls
                nc.vector.tensor_tensor(h8new[:, :, pbs], cand, e_sb, Alu.add)
            nc.vector.tensor_tensor(hnew[:, :, pbs], cand, e_sb, Alu.add)

        def emit_tail():
            emit_tail_head()
            emit_tail_combine()

        os_sb = {}   # (t, j) -> o_sb tile [33, n] (o - db at rows 0 and 32)

        def emit_dense_o(t, j, h_t):
            """Dense output o(t, j) = h(t, j-half) @ dw."""
            bs = slice(j * n, (j + 1) * n)
            po = phh.tile([1, n], f32, tag="phh")
            nc.tensor.matmul(po, dw_sb[:, 0:1], h_t[:, 0, bs],
                             start=True, stop=False)
            nc.tensor.matmul(po, dw_sb[:, 1:2], h_t[:, 1, bs],
                             start=False, stop=True)
            o_sb = opool.tile([33, n], bf, tag="osb")
            if ocast_act:
                nc.scalar.activation(o_sb[0:1, :], po, Act.Copy)
            else:
                nc.vector.tensor_copy(out=o_sb[0:1, :], in_=po)
            # partition-32 replica via DMA (free) for K=1 pairing
            nc.sync.dma_start(out=o_sb[32:33, :], in_=o_sb[0:1, :])
            nc.sync.dma_start(out=outT[t:t + 1, bs], in_=o_sb[0:1, :])
            os_sb[(t, j)] = o_sb

        h_hist = {-1: (h_prev, h8_prev)}
        for t in range(t_steps):
            h_new = hpool.tile([128, NCH, bl], bf, tag="h")
            if any8:
                h8_new = h8pool.tile([128, NCH, bl], f8, tag="h8")
            else:
                h8_new = h_new
            h_hist[t] = (h_new, h8_new)
            h_prev, h8_prev = h_hist[t - 1]
            for j in range(nt):
                bs = slice(j * n, (j + 1) * n)
                x = xs[(t, j)]

                # --- r + xh feature matmuls, row-tile paired (c0 hi / c1 lo)
                rp = pzr.tile([128, NCH, n], f32, tag="pzr")
                nc.tensor.matmul(rp[:, 0, :], kxf_sb[64:128, H:H + 128],
                                 x[64:128, :], start=True, stop=False)
                nc.tensor.matmul(rp[:, 1, :], kxf_sb[0:64, H + 128:2 * H],
                                 x[0:64, :], start=True, stop=False)
                xhp = pxh.tile([128, NCH, n], f32, tag="pxh")
                nc.tensor.matmul(xhp[:, 0, :], kxf_sb[64:128, 2 * H:2 * H + 128],
                                 x[64:128, :], start=True, stop=False)
                nc.tensor.matmul(xhp[:, 1, :], kxf_sb[0:64, 2 * H + 128:G3],
                                 x[0:64, :], start=True, stop=False)

                # --- dense output + o-cast of step t-1 (one step of slack;
                # early so its ocast heads the ACT queue) ---
                if t > 0:
                    emit_dense_o(t - 1, j, h_prev)

                # --- r recurrent matmuls + sigmoid ---
                h_mms(rp[:, 0, :], H, t, bs, r8)
                h_mms(rp[:, 1, :], H + 128, t, bs, r8)
                r_sb = rpool.tile([128, NCH, n], bf, tag="rsb")
                nc.scalar.activation(r_sb, rp, Act.Sigmoid, scale=sr)

                # --- previous block's chain tail (first half) ---
                emit_tail_head()

                # --- z matmuls (zr PSUM slot reused after sig_r read) ---
                zp = pzr.tile([128, NCH, n], f32, tag="pzr")
                nc.tensor.matmul(zp[:, 0, :], kxf_sb[64:128, 0:128],
                                 x[64:128, :], start=True, stop=False)
                nc.tensor.matmul(zp[:, 1, :], kxf_sb[0:64, 128:256],
                                 x[0:64, :], start=True, stop=False)
                h_mms(zp[:, 0, :], 0, t, bs, z8)
                h_mms(zp[:, 1, :], 128, t, bs, z8)
                z_sb = zpool.tile([128, NCH, n], bf, tag="zsb")
                nc.scalar.activation(z_sb, zp, Act.Sigmoid, scale=sz)

                # --- hh matmuls ---
                hhp = phh.tile([128, NCH, n], f32, tag="phh")
                if hh8:
                    nc.tensor.matmul(hhp[:, 0, :], wr8_sb[:, :, 2 * H:2 * H + 128],
                                     h8_prev[:, :, bs], start=True, stop=True,
                                     perf_mode=PerfMode.DoubleRow)
                    nc.tensor.matmul(hhp[:, 1, :], wr8_sb[:, :, 2 * H + 128:G3],
                                     h8_prev[:, :, bs], start=True, stop=True,
                                     perf_mode=PerfMode.DoubleRow)
                else:
                    for c in range(NCH):
                        m = 2 * H + c * 128
                        nc.tensor.matmul(hhp[:, c, :], wrb_sb[:, 0, m:m + 128],
                                         h_prev[:, 0, bs], start=True, stop=False)
                        nc.tensor.matmul(hhp[:, c, :], wrb_sb[:, 1, m:m + 128],
                                         h_prev[:, 1, bs], start=False, stop=True)

                # --- xh o-feedback: K=1 matmuls with o(t-1), row-paired ---
                if t == 0:
                    o_p0, o_p1 = o0_sb[0:1, bs], o0_sb[32:33, bs]
                else:
                    osb = os_sb[(t - 1, j)]
                    o_p0, o_p1 = osb[0:1, :], osb[32:33, :]
                nc.tensor.matmul(xhp[:, 0, :], k0t_sb[0:1, 2 * H:2 * H + 128],
                                 o_p0, start=False, stop=not ci_pe)
                nc.tensor.matmul(xhp[:, 1, :], k0t_sb[32:33, 2 * H + 128:G3],
                                 o_p1, start=False, stop=not ci_pe)

                # --- rh = hh * r (early in the DVE queue) ---
                rh_sb = rhpool.tile([128, NCH, n], bf, tag="rh")
                nc.vector.tensor_tensor(rh_sb, hhp, r_sb, Alu.mult)

                # --- previous block's combine ---
                emit_tail_combine()

                pending = (bs, xhp, rh_sb, z_sb, h_prev, h_new, h8_new)

                # --- prefetch next step's features ---
                if t < t_steps - 1:
                    xj = xpool.tile([128, n], bf, tag="x")
                    nc.sync.dma_start(
                        out=xj, in_=featT2[t + 1, :, j * n:(j + 1) * n])
                    xs[(t + 1, j)] = xj

            h_hist.pop(t - 2, None)
            xs.pop((t - 1, 0), None)
            xs.pop((t - 1, 1), None)
            for jj in range(nt):
                os_sb.pop((t - 2, jj), None)

        # final chain tail + last step's dense outputs
        emit_tail()
        for j in range(nt):
            emit_dense_o(t_steps - 1, j, h_hist[t_steps - 1][0])

    nc.compile()
    return nc


_NC_CACHE: dict = {}


def _flags():
    return dict(
        gates8=os.environ.get("V3_GATES8", "r"),
        ocast_act=os.environ.get("V3_OCAST_ACT", "1") == "1",
        ci_pe=os.environ.get("V3_CI_PE", "1") == "1",
    )


def _get_nc(t_steps=T, bl=BL, nt=2, **kw):
    flags = {**_flags(), **kw}
    key = (t_steps, bl, nt, tuple(sorted(flags.items())))
    if key not in _NC_CACHE:
        _NC_CACHE[key] = build_nc(t_steps, bl, nt, **flags)
    return _NC_CACHE[key]


def make_in_maps(
    decoder_feature, init_state, decoder_init_input, kernel, recurrent_kernel,
    input_bias, recurrent_bias, dense_w, dense_b,
    gates8="", t_steps=T, bl=BL, ncores=NCORES,
):
    bf_np = mybir.dt.np(mybir.dt.bfloat16)
    f8_np = mybir.dt.np(mybir.dt.float8e4)
    z8, r8, hh8 = ("z" in gates8), ("r" in gates8), ("h" in gates8)

    f = np.asarray(decoder_feature, np.float32)
    h0 = np.asarray(init_state, np.float32)
    o0 = np.asarray(decoder_init_input, np.float32)
    kx = np.asarray(kernel, np.float32)
    wr = np.asarray(recurrent_kernel, np.float32)
    ib = np.asarray(input_bias, np.float32)
    rb = np.asarray(recurrent_bias, np.float32)
    dw = np.asarray(dense_w, np.float32)
    db = float(np.asarray(dense_b, np.float32).reshape(-1)[0])
    assert not ib.any() and not rb.any() and db == 0.0, \
        "nonzero biases not supported by this kernel variant"
    k0 = kx[0]

    # o-feedback fold into the z/r columns of the recurrent weights
    wr_folded = wr.copy()
    wr_folded[:, :2 * H] += dw @ k0[None, :2 * H]

    # per-column scale: S8 for fp8 gates (their whole PSUM is scaled)
    s_col = np.ones(G3, np.float32)
    if z8:
        s_col[0:H] = S8
    if r8:
        s_col[H:2 * H] = S8
    if hh8:
        s_col[2 * H:] = S8

    kxf = np.empty((128, G3), np.float32)
    kxf[0:64] = kx[1:] * s_col
    kxf[64:128] = kx[1:] * s_col
    k0t = np.zeros((33, G3), np.float32)
    k0t[0] = k0 * s_col
    k0t[32] = k0 * s_col

    wrq = (wr_folded * s_col).reshape(2, 128, G3).transpose(1, 0, 2)
    wrz0 = (wr[:, :2 * H] * s_col[:2 * H]).reshape(2, 128, 2 * H)\
        .transpose(1, 0, 2)
    # bf16 copies use unscaled weights (scale only matters for fp8 psums)
    wrqb = wr_folded.reshape(2, 128, G3).transpose(1, 0, 2)
    wrz0b = wr[:, :2 * H].reshape(2, 128, 2 * H).transpose(1, 0, 2)

    in_maps = []
    for i in range(ncores):
        s = slice(i * bl, (i + 1) * bl)
        featT = f[s, :t_steps].transpose(1, 2, 0)        # [T, F, bl]
        featT2 = np.concatenate([featT, featT], axis=1)  # [T, 128, bl]
        h0T = h0[s].T.reshape(2, 128, bl).transpose(1, 0, 2)
        in_maps.append({
            "featT2": np.ascontiguousarray(featT2).astype(bf_np),
            "h0T": np.ascontiguousarray(h0T).astype(bf_np),
            "h08T": np.ascontiguousarray(h0T).astype(f8_np),
            "o0": np.ascontiguousarray((o0[s] - db).T).astype(bf_np),
            "kxf": kxf.astype(bf_np),
            "k0t": np.ascontiguousarray(k0t).astype(bf_np),
            "wrqb": np.ascontiguousarray(wrqb).astype(bf_np),
            "wrq8": np.ascontiguousarray(wrq).astype(f8_np),
            "wrz0b": np.ascontiguousarray(wrz0b).astype(bf_np),
            "wrz08": np.ascontiguousarray(wrz0).astype(f8_np),
            "dww": np.ascontiguousarray(dw.reshape(2, 128).T).astype(bf_np),
            "ident": np.eye(128, dtype=np.float32).astype(bf_np),
        })
    return in_maps, db


def run(inputs: dict, nt=2, trace=False, trace_kwargs=None, **kw):
    t_steps = int(inputs.get("predict_seq_length", T))
    assert t_steps == T, f"kernel hardcodes T={T}, got {t_steps}"
    flags = {**_flags(), **kw}
    nc = _get_nc(T, BL, nt, **flags)
    in_maps, db = make_in_maps(
        inputs["decoder_feature"], inputs["init_state"],
        inputs["decoder_init_input"], inputs["kernel"],
        inputs["recurrent_kernel"], inputs["input_bias"],
        inputs["recurrent_bias"], inputs["dense_w"], inputs["dense_b"],
        gates8=flags["gates8"],
    )
    res = bass_utils.run_bass_kernel_spmd(
        nc, in_maps, core_ids=list(range(NCORES)), trace=trace,
        **(trace_kwargs or {}),
    )
    out = np.empty((B, T, 1), np.float32)
    for i in range(NCORES):
        out[i * BL:(i + 1) * BL, :, 0] = \
            res.results[i]["outT"].astype(np.float32).T + db
    return out, res


def kernel(**inputs) -> np.ndarray:
    out, _ = run(inputs)
    return out
